# revision 54
# baseline (speedup 1.0000x reference)
"""GAT (4-layer graph attention network) on 8 Trainium2 NeuronCores.

Sharding (per hint): nodes in 8 contiguous ranges; edges partitioned by DST
node so edge-softmax + scatter-aggregation stay device-local.

v2 design (fp8 tables + streamed one-hot):
  - Per-layer DRAM gather tables hold fp8 rows [feats | s_src | s_dst]:
    layer 1 rows are 512B ([256 feats | 4 s_src | 4 s_dst | pad]), layers
    2-4 rows are 256B ([64 feats | s_src | s_dst | pad]).  Layer-1's table
    is built replicated (x@W1 is cheap); layers 2-4 build local rows and
    AllGather.
  - Per-edge source rows are fetched with the GPSIMD bulk gather
    (InstDMAGatherAnt, fp8 elem) in chunks sorted by dst on SWDGE queues
    1-3; the dst node's [s_src|s_dst] pair (4-8B) is fetched per edge with
    indirect_dma_start (queue 0) -- no 256B row floor.
  - The edge->dst one-hot S matrices are STATIC: precomputed on host as an
    fp8 DRAM table and streamed per supertile (no DVE is_equal build).
  - Scores: e = lrelu(s_src + s_dst) and p = exp(e) run on the Scalar
    engine (Lrelu/Exp activations); p is multiplied into the fp8 feats and
    written over the s_src column so each chunk's aggregation is a single
    fp8 matmul  ps[dst, f|den] += S_k^T @ V_k.
  - Final: per-graph mean-pool partials via one-hot batch matmul,
    AllReduce, replicated f32 MLP head.

kernel(**inputs) takes FULL inputs, returns the full [B, C] f32 output.
"""

import math
from contextlib import ExitStack

import numpy as np
import ml_dtypes

N_CORES = 8
NEG = 0.2
EPS = 1e-5
P = 128
DEF_G = 2          # dst-node tiles per gather "supertile"
DEF_SL = 4096      # xT streaming slab columns (dense phase)
SPLIT_CHUNKS = 8   # dma_gather call size (x128 idx)
SCRATCH = 65536    # dynamic DMA descriptor carveout bytes (4096 desc/queue)
EDGE_LEVEL = 2     # debug: 0=gathers only, 1=+scalar pipeline, 2=full

BF = ml_dtypes.bfloat16
F8 = ml_dtypes.float8_e4m3


def cdiv(a, b):
    return -(-a // b)


# ----------------------------------------------------------------------------
# Host-side planning / preprocessing
# ----------------------------------------------------------------------------

class Plan:
    """Static, core-independent program structure (cross-core maxima)."""

    def __init__(self, N, E, B, IN, HID, Hh, n_cores, half, G, edge_index):
        self.N, self.E, self.B, self.IN, self.HID, self.Hh = N, E, B, IN, HID, Hh
        self.n_cores = n_cores
        self.half = half
        self.G = G
        self.npc = N // n_cores                 # nodes per core
        self.T = cdiv(self.npc, P)              # dst tiles per core
        src = np.asarray(edge_index[0], np.int64)
        dst = np.asarray(edge_index[1], np.int64)
        order = np.argsort(dst, kind="stable")
        self.src_s = src[order].astype(np.int32)
        self.dst_s = dst[order].astype(np.int32)

        npc, T, n = self.npc, self.T, n_cores
        self.tile_edges = [[None] * T for _ in range(n)]
        k_lo = np.zeros((n, T), np.int64)
        k_hi = np.zeros((n, T), np.int64)
        for c in range(n):
            base = c * npc
            for t in range(T):
                lo_n = base + t * P
                hi_n = min(base + (t + 1) * P, base + npc)
                a = int(np.searchsorted(self.dst_s, lo_n))
                b = int(np.searchsorted(self.dst_s, hi_n))
                lo_m = self.src_s[a:b] < half
                self.tile_edges[c][t] = (a, b, lo_m)
                k_lo[c, t] = cdiv(int(lo_m.sum()), P)
                k_hi[c, t] = cdiv(int((~lo_m).sum()), P)
        self.K_lo = np.maximum(k_lo.max(axis=0), 1).astype(np.int64)   # >=1
        self.K_hi = k_hi.max(axis=0).astype(np.int64)                  # may be 0

        self.sts = [(s, min(s + G, T)) for s in range(0, T, G)]
        self.st_lo = [int(self.K_lo[a:b].sum()) for a, b in self.sts]
        self.st_hi = [int(self.K_hi[a:b].sum()) for a, b in self.sts]
        self.st_K = [l + h for l, h in zip(self.st_lo, self.st_hi)]
        self.stoff = np.concatenate([[0], np.cumsum(self.st_K)]).astype(np.int64)
        self.TC = int(self.stoff[-1])                   # total chunks
        self.Kmax = max(self.st_K)

        # chunk columns (within supertile) for each tile
        self.tile_cols = {t: [] for t in range(T)}
        for si, (a, b) in enumerate(self.sts):
            off = 0
            for t in range(a, b):
                self.tile_cols[t].append(("lo", si, off, int(self.K_lo[t])))
                off += int(self.K_lo[t])
            for t in range(a, b):
                if self.K_hi[t]:
                    self.tile_cols[t].append(("hi", si, off, int(self.K_hi[t])))
                off += int(self.K_hi[t])

        # gather-idx column offsets (int16 cols = n/16) per (st, half)
        self.g_off = []
        go = 0
        for si in range(len(self.sts)):
            lo_cols = 8 * self.st_lo[si]
            hi_cols = 8 * self.st_hi[si]
            self.g_off.append((go, lo_cols, go + lo_cols, hi_cols))
            go += lo_cols + hi_cols
        self.GCOLS = max(go, 1)


def _wrap16(vals16):
    """[n] -> [128, n/16] int16: 16-partition-wrapped, replicated x8."""
    n = vals16.shape[0]
    assert n % 16 == 0
    a = vals16.reshape(n // 16, 16).T.astype(np.int16)
    return np.tile(a, (8, 1))


def preprocess(inputs, n_cores=N_CORES, half=None, G=DEF_G, B=None):
    x = np.asarray(inputs["x"], np.float32)
    edge_index = np.asarray(inputs["edge_index"])
    batch = np.asarray(inputs["batch"], np.int64)
    N, IN = x.shape
    E = edge_index.shape[1]
    a_src1 = np.asarray(inputs["a_src1"], np.float32)
    Hh, HID = a_src1.shape
    C = np.asarray(inputs["Wh2"], np.float32).shape[1]
    if B is None:
        B = 64 if N == 50000 else int(batch.max()) + 1
    if half is None:
        half = N if N <= 32768 else (N + 1) // 2
    assert half <= 32768 and (N - half) <= 32768

    plan = Plan(N, E, B, IN, HID, Hh, n_cores, half, G, edge_index)
    npc, T = plan.npc, plan.T

    HF = Hh * HID                               # layer-1 out features (256)
    R1 = 512                                    # layer-1 row elems (fp8)
    R2 = 256                                    # layer 2-4 row elems (fp8)

    def fold(W, a_s, a_d):
        W = np.asarray(W, np.float32)
        a_s = np.asarray(a_s, np.float32)
        a_d = np.asarray(a_d, np.float32)
        Fin = W.shape[0]
        hh, F = a_s.shape
        Wr = W.reshape(Fin, hh, F)
        ws = np.einsum("ihf,hf->ih", Wr, a_s)
        wd = np.einsum("ihf,hf->ih", Wr, a_d)
        return np.concatenate([W, ws, wd], axis=1).astype(BF)

    w1p = fold(inputs["W1"], a_src1, inputs["a_dst1"])
    w2p = fold(inputs["W2"], inputs["a_src2"], inputs["a_dst2"])
    # [HF, HID+2] -> [128, (HF//128)*(HID+2)]  (contraction blocks side by side)
    nq2 = HF // P
    w2p = np.concatenate([w2p[q * P:(q + 1) * P, :] for q in range(nq2)],
                         axis=1)
    w3p = fold(inputs["W3"], inputs["a_src3"], inputs["a_dst3"])
    w4p = fold(inputs["W4"], inputs["a_src4"], inputs["a_dst4"])

    b1rep = np.tile(np.asarray(inputs["b1"], np.float32)[None, :], (P, 1))
    gs = 1.0 / math.sqrt(1.0 + EPS)

    def bn_fold(g, b, be):
        gg = np.asarray(g, np.float32) * gs
        bb = gg * np.asarray(b, np.float32) + np.asarray(be, np.float32)
        return (np.tile(gg[None, :], (P, 1)).astype(np.float32),
                np.tile(bb[None, :], (P, 1)).astype(np.float32))

    gg2, bb2 = bn_fold(inputs["g2"], inputs["b2"], inputs["be2"])
    gg3, bb3 = bn_fold(inputs["g3"], inputs["b3"], inputs["be3"])
    gg4, bb4 = bn_fold(inputs["g4"], inputs["b4"], inputs["be4"])

    wh1 = np.asarray(inputs["Wh1"], np.float32)
    MH = wh1.shape[1]
    bh1rep = np.tile(np.asarray(inputs["bh1"], np.float32)[None, :], (B, 1))
    wh2 = np.asarray(inputs["Wh2"], np.float32)
    bh2rep = np.tile(np.asarray(inputs["bh2"], np.float32)[None, :], (B, 1))

    xT = np.ascontiguousarray(x.T).astype(BF)
    idbf = np.eye(P, dtype=np.float32).astype(BF)
    idf32 = np.eye(P, dtype=np.float32)
    iota = np.tile(np.arange(P, dtype=np.float32)[None, :], (P, 1)).astype(BF)
    onescol = np.ones((P, 1), np.float32).astype(BF)

    common = dict(xT=xT, w1p=w1p, w2p=w2p, w3p=w3p, w4p=w4p, b1rep=b1rep,
                  gg2=gg2, bb2=bb2, gg3=gg3, bb3=bb3, gg4=gg4, bb4=bb4,
                  wh1=wh1, bh1rep=bh1rep, wh2=wh2, bh2rep=bh2rep,
                  idbf=idbf, idf32=idf32, iota=iota, onescol=onescol)

    # per-node layer-1 dst scores (a host-side weight-fold product): the
    # kernel reads the dst tile's scores as a tiny constant instead of
    # re-extracting them from the replicated table at a core-dependent
    # address.
    wd1 = np.einsum("ihf,hf->ih", np.asarray(inputs["W1"], np.float32)
                    .astype(BF).astype(np.float32).reshape(IN, Hh, HID),
                    np.asarray(inputs["a_dst1"], np.float32))
    sdst1_all = (x.astype(BF).astype(np.float32) @ wd1).astype(F8)  # [N, Hh]

    per_core = []
    for c in range(n_cores):
        base = c * npc
        gidx = np.zeros((128, plan.GCOLS), np.int16)
        sdime = np.zeros((128, max(plan.TC, 1) * P), F8)
        sdimeT = np.zeros((128, max(plan.TC, 1) * P), F8)
        for si, (a, b) in enumerate(plan.sts):
            glo, glo_n, ghi, ghi_n = plan.g_off[si]
            lo_vals = np.zeros(16 * glo_n, np.int16)
            hi_vals = np.zeros(16 * ghi_n, np.int16)
            for t in range(a, b):
                ea, eb, lo_m = plan.tile_edges[c][t]
                s_all = plan.src_s[ea:eb]
                d_all = plan.dst_s[ea:eb]
                for kind, tsi, off, K in plan.tile_cols[t]:
                    if tsi != si:
                        continue
                    sel = lo_m if kind == "lo" else ~lo_m
                    vals = s_all[sel] - (0 if kind == "lo" else half)
                    dl = d_all[sel] - (base + t * P)   # dst idx within tile
                    m = vals.shape[0]
                    npad = K * P
                    v = np.zeros(npad, np.int16)
                    v[:m] = vals.astype(np.int16)
                    if kind == "lo":
                        lo_vals[off * P: off * P + npad] = v
                    else:
                        ho = off - plan.st_lo[si]
                        hi_vals[ho * P: ho * P + npad] = v
                    # one-hot S (edge-major) and ST (dst-major) columns
                    dv = np.full(npad, -1, np.int64)
                    dv[:m] = dl
                    for k in range(K):
                        col0 = int(plan.stoff[si]) + off + k
                        dvk = dv[k * P:(k + 1) * P]
                        S = np.zeros((P, P), np.float32)
                        valid = dvk >= 0
                        S[np.arange(P)[valid], dvk[valid]] = 1.0
                        sdime[:, col0 * P:(col0 + 1) * P] = S.astype(F8)
                        sdimeT[:, col0 * P:(col0 + 1) * P] = \
                            np.ascontiguousarray(S.T).astype(F8)
            if glo_n:
                gidx[:, glo:glo + glo_n] = _wrap16(lo_vals)
            if ghi_n:
                gidx[:, ghi:ghi + ghi_n] = _wrap16(hi_vals)

        batchv = np.full((128, T), -1.0, np.float32)
        sdv1 = np.zeros((128, T, Hh), F8)
        for t in range(T):
            lo_n = base + t * P
            hi_n = min(base + (t + 1) * P, base + npc)
            batchv[: hi_n - lo_n, t] = batch[lo_n:hi_n].astype(np.float32)
            sdv1[: hi_n - lo_n, t, :] = sdst1_all[lo_n:hi_n]

        per_core.append(dict(gidx=gidx, sdime=sdime, sdimeT=sdimeT,
                             sdv1=sdv1.reshape(128, T * Hh),
                             batchv=batchv.astype(BF)))

    meta = dict(plan=plan, HF=HF, R1=R1, R2=R2, C=C, MH=MH, B=B)
    return meta, common, per_core


# ----------------------------------------------------------------------------
# Bass program (shared by all cores; per-core behavior differs only via data)
# ----------------------------------------------------------------------------

_QPATCH = [False]


def _install_queue_patch():
    """Rewrite each Pool-DMA instruction's SWDGE queue to (sem lane % 4).

    The tile framework assigns DMASW sem lanes round-robin over Pool DMA
    instructions in POST-SCHEDULE order; each sem lane must be serviced by a
    single queue.  Queue choice at emission time cannot guarantee that (the
    scheduler reorders), so fix the queue inside the tick-assignment pass
    where the lane is known.
    """
    if _QPATCH[0]:
        return
    import concourse.mybir as mybir
    import concourse.bass_isa as bass_isa
    import concourse.tile_sem_assignment as tsa

    orig = tsa.TileClockTick._assign_tick

    def patched(self, inst):
        if (isinstance(inst, tsa.DMAInst)
                and not isinstance(inst, bass_isa.UserSyncedRemoteDMADescs)
                and inst.engine == mybir.EngineType.Pool):
            if isinstance(inst, mybir.InstDMACopy):
                # indirect copies: dedicated sem lane 0 on queue 0 (the rust
                # sim pins InstDMACopy to qPoolDynamic queue 0 regardless of
                # the queue-name suffix)
                lane = 0
            else:
                ctr = getattr(self, "_gat_ctr", 0)
                lane = 1 + ctr % 7
                self._gat_ctr = ctr + 1
                inst.queue_num = lane % 4
            self.next_sw_dma_idx = lane  # orig consumes this as the lane
        return orig(self, inst)

    tsa.TileClockTick._assign_tick = patched
    _QPATCH[0] = True


def build_program(meta, phases=None, debug_dumps=False):
    import concourse.bass as bass
    import concourse.bacc as bacc
    import concourse.mybir as mybir
    import concourse.tile as tile

    _install_queue_patch()

    F32 = mybir.dt.float32
    BF16 = mybir.dt.bfloat16
    FP8 = mybir.dt.float8e4
    I16 = mybir.dt.int16
    I32 = mybir.dt.int32
    A = mybir.AluOpType
    ACT = mybir.ActivationFunctionType

    if phases is None:
        phases = ["dense", "e1", "ag1", "e2", "ag2", "e3", "ag3", "e4", "fin"]
    plan = meta["plan"]
    N, IN, Hh, HID = plan.N, plan.IN, plan.Hh, plan.HID
    B, C, MH = meta["B"], meta["C"], meta["MH"]
    HF, R1, R2 = meta["HF"], meta["R1"], meta["R2"]
    npc, T, half = plan.npc, plan.T, plan.half
    n_cores = plan.n_cores
    SL = min(DEF_SL, N)

    nc = bacc.Bacc("TRN2", num_devices=n_cores, num_swdge_queues=4,
                   dynamic_dma_scratch_size=SCRATCH)
    rg = [list(range(n_cores))]

    def ein(name, shape, dt):
        return nc.dram_tensor(name, shape, dt, kind="ExternalInput")

    xT_d = ein("xT", [IN, N], BF16)
    w1p_d = ein("w1p", [IN, HF + 2 * Hh], BF16)
    w2p_d = ein("w2p", [P, (HF // P) * (HID + 2)], BF16)
    w3p_d = ein("w3p", [HID, HID + 2], BF16)
    w4p_d = ein("w4p", [HID, HID + 2], BF16)
    b1rep_d = ein("b1rep", [P, HF], F32)
    gg_d = [None, ein("gg2", [P, HID], F32), ein("gg3", [P, HID], F32),
            ein("gg4", [P, HID], F32)]
    bb_d = [None, ein("bb2", [P, HID], F32), ein("bb3", [P, HID], F32),
            ein("bb4", [P, HID], F32)]
    wh1_d = ein("wh1", [HID, MH], F32)
    bh1rep_d = ein("bh1rep", [B, MH], F32)
    wh2_d = ein("wh2", [MH, C], F32)
    bh2rep_d = ein("bh2rep", [B, C], F32)
    idbf_d = ein("idbf", [P, P], BF16)
    idf32_d = ein("idf32", [P, P], F32)
    iota_d = ein("iota", [P, P], BF16)
    ones_d = ein("onescol", [P, 1], BF16)
    gidx_d = ein("gidx", [P, plan.GCOLS], I16)
    sdime_d = ein("sdime", [P, max(plan.TC, 1) * P], FP8)
    sdimeT_d = ein("sdimeT", [P, max(plan.TC, 1) * P], FP8)
    sdv1_d = ein("sdv1", [P, T * Hh], FP8)
    batchv_d = ein("batchv", [P, T], BF16)

    shr = "Shared" if n_cores > 4 else "Local"
    table1 = nc.dram_tensor("table1", [N, R1], FP8)
    tloc = [None, nc.dram_tensor("tloc2", [npc, R2], FP8),
            nc.dram_tensor("tloc3", [npc, R2], FP8),
            nc.dram_tensor("tloc4", [npc, R2], FP8)]
    tfull = [None,
             nc.dram_tensor("tfull2", [N, R2], FP8, addr_space=shr),
             nc.dram_tensor("tfull3", [N, R2], FP8, addr_space=shr),
             nc.dram_tensor("tfull4", [N, R2], FP8, addr_space=shr)]
    arin = nc.dram_tensor("arin", [HID, B + 1], F32)
    arout = nc.dram_tensor("arout", [HID, B + 1], F32, addr_space=shr)
    out_d = nc.dram_tensor("out", [B, C], F32, kind="ExternalOutput")
    dbg = {}
    if debug_dumps:
        dbg["t1"] = nc.dram_tensor("dbg_t1", [P, R1], FP8,
                                   kind="ExternalOutput")
        dbg["V"] = nc.dram_tensor("dbg_V", [P, R1], FP8,
                                  kind="ExternalOutput")
        dbg["ED"] = nc.dram_tensor("dbg_ED", [P, 16 * 8], F32,
                                   kind="ExternalOutput")
        dbg["x1"] = nc.dram_tensor("dbg_x1", [P, HF], F32,
                                   kind="ExternalOutput")
        dbg["S"] = nc.dram_tensor("dbg_S", [P, P], FP8,
                                  kind="ExternalOutput")

    gcnt = nc.gpsimd.alloc_register("gcnt")
    gcnt_cur = [-1]
    qctr = [0]

    def gather_split(out3, tab_ap, idx_sb, col0, n_chunks, elem, name):
        # split into SPLIT_CHUNKS (x128-idx) calls; queue_num is overwritten
        # post-schedule by _install_queue_patch.
        done = 0
        while done < n_chunks:
            nn = min(SPLIT_CHUNKS, n_chunks - done)
            if gcnt_cur[0] != nn * P:
                nc.gpsimd.reg_mov(gcnt, nn * P)
                gcnt_cur[0] = nn * P
            nc.gpsimd.dma_gather(
                out3[:, done:done + nn, :], tab_ap,
                idx_sb[:, col0 + 8 * done: col0 + 8 * (done + nn)],
                nn * P, gcnt, elem, queue_num=0)
            done += nn

    with ExitStack() as ctx:
        tc = ctx.enter_context(tile.TileContext(nc))
        cst = ctx.enter_context(tc.tile_pool(name="cst", bufs=1))
        vpool = ctx.enter_context(tc.tile_pool(name="vpool", bufs=3))
        edpool = ctx.enter_context(tc.tile_pool(name="edpool", bufs=6))
        sppool = ctx.enter_context(tc.tile_pool(name="sppool", bufs=3))
        fpool = ctx.enter_context(tc.tile_pool(name="fpool", bufs=2))
        hpool = ctx.enter_context(tc.tile_pool(name="hpool", bufs=1))
        xpool = ctx.enter_context(tc.tile_pool(name="xpool", bufs=2))
        wpool = ctx.enter_context(tc.tile_pool(name="wpool", bufs=2))
        ppool = ctx.enter_context(tc.tile_pool(name="ppool", bufs=2, space="PSUM"))
        epool = ctx.enter_context(tc.tile_pool(name="epool", bufs=2, space="PSUM"))
        tpool = ctx.enter_context(tc.tile_pool(name="tpool", bufs=1, space="PSUM"))

        def load_const(dram, shape, dt, name):
            t = cst.tile(shape, dt, name=name, tag=name)
            nc.sync.dma_start(out=t[:], in_=dram[:])
            return t

        w1p_s = load_const(w1p_d, [IN, HF + 2 * Hh], BF16, "w1p_s")
        w2p_s = load_const(w2p_d, [P, (HF // P) * (HID + 2)], BF16, "w2p_s")
        w3p_s = load_const(w3p_d, [HID, HID + 2], BF16, "w3p_s")
        w4p_s = load_const(w4p_d, [HID, HID + 2], BF16, "w4p_s")
        wlp_s = [None, w2p_s, w3p_s, w4p_s]
        b1rep_s = load_const(b1rep_d, [P, HF], F32, "b1rep_s")
        gg_s = [None] + [load_const(gg_d[i], [P, HID], F32, f"gg{i+1}_s")
                         for i in (1, 2, 3)]
        bb_s = [None] + [load_const(bb_d[i], [P, HID], F32, f"bb{i+1}_s")
                         for i in (1, 2, 3)]
        wh1_s = load_const(wh1_d, [HID, MH], F32, "wh1_s")
        bh1rep_s = load_const(bh1rep_d, [B, MH], F32, "bh1rep_s")
        wh2_s = load_const(wh2_d, [MH, C], F32, "wh2_s")
        bh2rep_s = load_const(bh2rep_d, [B, C], F32, "bh2rep_s")
        idbf_s = load_const(idbf_d, [P, P], BF16, "idbf_s")
        idf32_s = load_const(idf32_d, [P, P], F32, "idf32_s")
        iota_s = load_const(iota_d, [P, P], BF16, "iota_s")
        ones_s = load_const(ones_d, [P, 1], BF16, "ones_s")
        gidx_s = load_const(gidx_d, [P, plan.GCOLS], I16, "gidx_s")
        sdv1_s = load_const(sdv1_d, [P, T * Hh], FP8, "sdv1_s")
        batchv_s = load_const(batchv_d, [P, T], BF16, "batchv_s")

        # ---------------- layer-1 dense: replicated table1 build -----------
        W1C = HF + 2 * Hh                       # 264 written cols per row
        for sb in range(cdiv(N, SL) if "dense" in phases else 0):
            c0 = sb * SL
            c1 = min(c0 + SL, N)
            nblk = cdiv(c1 - c0, P)
            nfull = (c1 - c0) // P
            xsl = xpool.tile([IN, SL], BF16, tag="xsl", name=f"xsl{sb}")
            nc.sync.dma_start(out=xsl[:, 0:c1 - c0], in_=xT_d[:, c0:c1])
            stg = wpool.tile([P, cdiv(SL, P), W1C], FP8, tag="stg",
                             name=f"stg{sb}")
            for bi in range(nblk):
                b0 = c0 + bi * P
                b1_ = min(b0 + P, N)
                nb = b1_ - b0
                ps = ppool.tile([P, W1C], F32, tag="pU", name=f"psd{sb}_{bi}")
                nc.tensor.matmul(ps[:nb, :], lhsT=xsl[:, b0 - c0:b1_ - c0],
                                 rhs=w1p_s[:], start=True, stop=True)
                if bi % 2 == 0:
                    nc.vector.tensor_copy(stg[:nb, bi, :], ps[:nb, :])
                else:
                    nc.scalar.activation(out=stg[:nb, bi, :], in_=ps[:nb, :],
                                         func=ACT.Copy)
            if nfull:
                nc.sync.dma_start(
                    out=table1[c0:c0 + nfull * P, 0:W1C].rearrange(
                        "(b p) c -> p b c", p=P),
                    in_=stg[:, 0:nfull, :])
            if nfull < nblk:                      # tail rows (<128)
                nb = (c1 - c0) - nfull * P
                nc.sync.dma_start(
                    out=table1[c0 + nfull * P:c1, 0:W1C],
                    in_=stg[0:nb, nfull, :])

        # persistent residual-state tiles
        h_keep = {2: [], 3: []}
        for t in range(T):
            h_keep[2].append(hpool.tile([P, HID], BF16, tag=f"h2_{t}",
                                        name=f"h2_{t}"))
            h_keep[3].append(hpool.tile([P, HID], BF16, tag=f"h3_{t}",
                                        name=f"h3_{t}"))

        psA, _freeA = tc.tile([HID, B], F32, space="PSUM", name="psA")
        psB, _freeB = tc.tile([B, 1], F32, space="PSUM", name="psB")

        def _tile_ranges(si):
            ranges = {}
            for t in range(*plan.sts[si]):
                for kind, tsi, off, K in plan.tile_cols[t]:
                    if tsi == si and K:
                        ranges.setdefault(t, []).append((off, off + K))
            out = []
            for t in sorted(ranges):
                for a, b in ranges[t]:
                    if out and out[-1][1] == a:
                        out[-1] = (out[-1][0], b)
                    else:
                        out.append((a, b))
            return out

        # ---------------- edge phase (layers 1..4) ----------------
        def edge_phase(l):
            """l in 1..4 (1-indexed); software-pipelined: fetch supertile
            si+PF (gathers + S/ST streams) while computing supertile si, so
            every engine's in-order stream has DMA issued PF supertiles
            ahead of the dependent compute."""
            if l == 1:
                R, HFl, Hl = R1, HF, Hh
                tab = table1
            else:
                R, HFl, Hl = R2, HID, 1
                tab = tfull[l - 1]

            def fetch(si):
                K_st = plan.st_K[si]
                lo_c = plan.st_lo[si]
                hi_c = plan.st_hi[si]
                c0 = int(plan.stoff[si])
                V = vpool.tile([P, K_st, R], FP8, tag="V", name=f"V{l}_{si}")
                glo, glo_n, ghi, ghi_n = plan.g_off[si]
                if lo_c and EDGE_LEVEL != -1:
                    gather_split(V, tab[0:half, 0:R], gidx_s, glo, lo_c, R,
                                 f"glo{l}_{si}")
                if hi_c and EDGE_LEVEL != -1:
                    gather_split(V[:, lo_c:K_st, :], tab[half:N, 0:R],
                                 gidx_s, ghi, hi_c, R, f"ghi{l}_{si}")
                S_sl = sppool.tile([P, K_st * P], FP8, tag="S",
                                   name=f"S{l}_{si}")
                nc.sync.dma_start(out=S_sl[:],
                                  in_=sdime_d[:, c0 * P:(c0 + K_st) * P])
                ST_sl = sppool.tile([P, K_st * P], FP8, tag="ST",
                                    name=f"ST{l}_{si}")
                nc.sync.dma_start(out=ST_sl[:],
                                  in_=sdimeT_d[:, c0 * P:(c0 + K_st) * P])
                sdcs = {}
                if l > 1:
                    for t in range(*plan.sts[si]):
                        r0 = t * P
                        nt = min(r0 + P, npc) - r0
                        sdt = edpool.tile([P, Hl], FP8, tag="sdc",
                                          name=f"sdc{l}_{t}")
                        if nt < P:
                            nc.vector.memset(sdt[:], 0.0)
                        nc.sync.dma_start(
                            out=sdt[0:nt, :],
                            in_=tloc[l - 1][r0:r0 + nt, HID + 1:HID + 2])
                        sdcs[t] = sdt
                return dict(V=V, S=S_sl, ST=ST_sl, sdcs=sdcs)

            def compute(si, hd):
                ta, tb_ = plan.sts[si]
                K_st = plan.st_K[si]
                c0 = int(plan.stoff[si])
                V, S_sl, ST_sl = hd["V"], hd["S"], hd["ST"]
                if EDGE_LEVEL == -2 or EDGE_LEVEL == -1:
                    return
                # per-edge s_dst via one-hot matmul: psE[e,:] = ST_k^T @ sdc
                psE = epool.tile([P, K_st * Hl], F32, tag="pE",
                                 name=f"pE{l}_{si}")
                for t in range(ta, tb_):
                    if l == 1:
                        sdc = sdv1_s[:, t * Hh:(t + 1) * Hh]
                    else:
                        sdc = hd["sdcs"][t][:]
                    for kind, tsi, off, K in plan.tile_cols[t]:
                        if tsi != si:
                            continue
                        for k in range(off, off + K):
                            nc.tensor.matmul(
                                psE[:, k * Hl:(k + 1) * Hl],
                                lhsT=ST_sl[:, k * P:(k + 1) * P],
                                rhs=sdc, start=True, stop=True)

                if EDGE_LEVEL < 1:
                    return
                # scores: e = lrelu(s_src + s_dst); p = exp(e)
                e_t = fpool.tile([P, K_st * Hl], F32, tag="e_t",
                                 name=f"e{l}_{si}")
                ev = e_t[:].rearrange("p (k h) -> p k h", h=Hl)
                pEv = psE[:].rearrange("p (k h) -> p k h", h=Hl)
                nc.vector.tensor_tensor(
                    out=ev, in0=V[:, :, HFl:HFl + Hl],
                    in1=pEv, op=A.add)
                # p = exp(lrelu(e)) == max(exp(e), exp(NEG*e)): both exps on
                # the (idle) Scalar engine, one DVE max
                tmp_t = fpool.tile([P, K_st * Hl], F32, tag="tmp_t",
                                   name=f"tmp{l}_{si}")
                nc.scalar.activation(out=tmp_t[:], in_=e_t[:], func=ACT.Exp,
                                     scale=NEG)
                p_t = fpool.tile([P, K_st * Hl], F32, tag="p_t",
                                 name=f"p{l}_{si}")
                nc.scalar.activation(out=p_t[:], in_=e_t[:], func=ACT.Exp)
                nc.vector.tensor_tensor(out=p_t[:], in0=p_t[:], in1=tmp_t[:],
                                        op=A.max)
                pv = p_t[:].rearrange("p (k h) -> p k h", h=Hl)
                # features *= p (in place, fp8), then p -> s_src column;
                # issued per chunk-range so each tile's aggregation can
                # start as soon as its own columns are scaled
                for ca, cb in _tile_ranges(si):
                    kv = cb - ca
                    v4 = V[:, ca:cb, 0:HFl].rearrange(
                        "p k (h f) -> p k h f", f=HID)
                    pb = pv[:, ca:cb].unsqueeze(3).to_broadcast(
                        [P, kv, Hl, HID])
                    nc.vector.tensor_tensor(out=v4, in0=v4, in1=pb,
                                            op=A.mult)
                    nc.scalar.activation(out=V[:, ca:cb, HFl:HFl + Hl],
                                         in_=pv[:, ca:cb], func=ACT.Copy)

                if EDGE_LEVEL < 2:
                    return
                for t in range(ta, tb_):
                    cols = []
                    for kind, tsi, off, K in plan.tile_cols[t]:
                        if tsi == si:
                            cols += list(range(off, off + K))
                    ps = ppool.tile([P, HFl + Hl], F32, tag="pU",
                                    name=f"pU{l}_{t}")
                    for j, k in enumerate(cols):
                        nc.tensor.matmul(ps[:], lhsT=S_sl[:, k * P:(k + 1) * P],
                                         rhs=V[:, k, 0:HFl + Hl],
                                         start=(j == 0),
                                         stop=(j == len(cols) - 1))
                    finalize(l, t, ps, HFl, Hl)

            PF = 2
            nst = len(plan.sts)
            hs = {}
            for si in range(nst + PF):
                if si < nst:
                    hs[si] = fetch(si)
                if si >= PF:
                    compute(si - PF, hs.pop(si - PF))

        def finalize(l, t, ps, HFl, Hl):
            r0 = t * P
            r1 = min(r0 + P, npc)
            nt = r1 - r0
            dm = fpool.tile([P, Hl], F32, tag="dm", name=f"dm{l}_{t}")
            nc.vector.tensor_scalar(dm[:], ps[:, HFl:HFl + Hl], 1e-16, None,
                                    A.max)
            rc = fpool.tile([P, Hl], F32, tag="rc", name=f"rc{l}_{t}")
            nc.vector.reciprocal(rc[:], dm[:])
            if l == 1:
                y = fpool.tile([P, HFl], F32, tag="y1", name=f"y1_{t}")
                y4 = y[:].rearrange("p (h f) -> p h f", f=HID)
                u4 = ps[:, 0:HFl].rearrange("p (h f) -> p h f", f=HID)
                rb = rc[:].unsqueeze(2).to_broadcast([P, Hl, HID])
                nc.vector.tensor_tensor(out=y4, in0=u4, in1=rb, op=A.mult)
                nc.vector.tensor_tensor(out=y[:], in0=y[:], in1=b1rep_s[:],
                                        op=A.add)
                x1 = fpool.tile([P, HFl], BF16, tag="x1", name=f"x1_{t}")
                nc.vector.tensor_scalar(x1[:], y[:], 0.0, None, A.max)
                if debug_dumps and t == 0:
                    nc.sync.dma_start(out=dbg["x1"][:], in_=y[:])
                # next table rows: tloc2 = fp8(x1 @ w2p)
                pt2 = tpool.tile([P, HID + 2], F32, tag="tN", name=f"pt2_{t}")
                nq = HF // P
                for q in range(nq):
                    pT = tpool.tile([P, P], BF16, tag="tT", name=f"pT{t}_{q}")
                    nc.tensor.transpose(pT[:], x1[:, q * P:(q + 1) * P],
                                        idbf_s[:])
                    sT = fpool.tile([P, P], BF16, tag="sT", name=f"sT{t}_{q}")
                    nc.scalar.activation(out=sT[:], in_=pT[:], func=ACT.Copy)
                    nc.tensor.matmul(pt2[:nt, :], lhsT=sT[:, 0:nt],
                                     rhs=w2p_s[:, q * (HID + 2):
                                               (q + 1) * (HID + 2)],
                                     start=(q == 0), stop=(q == nq - 1))
                tb2 = fpool.tile([P, HID + 2], FP8, tag="tb2",
                                 name=f"tb2_{t}")
                nc.scalar.activation(out=tb2[:nt, :], in_=pt2[:nt, :],
                                     func=ACT.Copy)
                nc.sync.dma_start(out=tloc[1][r0:r1, 0:HID + 2],
                                  in_=tb2[:nt, :])
            else:
                y = fpool.tile([P, HID], F32, tag="y2", name=f"y2{l}_{t}")
                nc.vector.scalar_tensor_tensor(
                    out=y[:], in0=ps[:, 0:HID], scalar=rc[:, 0:1],
                    in1=gg_s[l - 1][:], op0=A.mult, op1=A.mult)
                nc.vector.tensor_tensor(out=y[:], in0=y[:],
                                        in1=bb_s[l - 1][:], op=A.add)
                if l == 2:
                    hn = h_keep[2][t]
                    nc.vector.tensor_scalar(hn[:], y[:], 0.0, None, A.max)
                else:
                    nc.vector.tensor_scalar(y[:], y[:], 0.0, None, A.max)
                    prev = h_keep[l - 1][t]
                    hn = h_keep[3][t] if l == 3 else \
                        fpool.tile([P, HID], BF16, tag="h4", name=f"h4_{t}")
                    nc.vector.tensor_tensor(out=hn[:], in0=y[:], in1=prev[:],
                                            op=A.add)
                if l < 4:
                    # next table rows: tloc_{l+1} = fp8(hn @ w_{l+1}p)
                    pT = tpool.tile([HID, P], BF16, tag="tT",
                                    name=f"pTh{l}_{t}")
                    nc.tensor.transpose(pT[:], hn[:], idbf_s[:])
                    sT = fpool.tile([HID, P], BF16, tag="sTh",
                                    name=f"sTh{l}_{t}")
                    nc.scalar.activation(out=sT[:], in_=pT[:], func=ACT.Copy)
                    ptn = tpool.tile([P, HID + 2], F32, tag="tN",
                                     name=f"ptn{l}_{t}")
                    nc.tensor.matmul(ptn[:nt, :], lhsT=sT[:, 0:nt],
                                     rhs=wlp_s[l][:], start=True, stop=True)
                    tbn = fpool.tile([P, HID + 2], FP8, tag="tbn",
                                     name=f"tbn{l}_{t}")
                    nc.scalar.activation(out=tbn[:nt, :], in_=ptn[:nt, :],
                                         func=ACT.Copy)
                    nc.sync.dma_start(out=tloc[l][r0:r1, 0:HID + 2],
                                      in_=tbn[:nt, :])
                else:
                    # pooling partials
                    Sb = fpool.tile([P, B], BF16, tag="Sb", name=f"Sb_{t}")
                    bv = batchv_s[:, t:t + 1].to_broadcast([P, B])
                    nc.vector.tensor_tensor(out=Sb[:], in0=iota_s[:, 0:B],
                                            in1=bv, op=A.is_equal)
                    nc.tensor.matmul(psA[:], lhsT=hn[:], rhs=Sb[:],
                                     start=(t == 0), stop=(t == T - 1))
                    nc.tensor.matmul(psB[:], lhsT=Sb[:], rhs=ones_s[:],
                                     start=(t == 0), stop=(t == T - 1))

        def all_gather(l):
            if n_cores == 1:
                nc.sync.dma_start(out=tfull[l][:], in_=tloc[l][:])
            else:
                nc.gpsimd.collective_compute(
                    "AllGather", A.bypass, replica_groups=rg,
                    ins=[tloc[l][:]], outs=[tfull[l][:]])

        if "e1" in phases:
            edge_phase(1)
        if "ag1" in phases:
            all_gather(1)
        if "e2" in phases:
            edge_phase(2)
        if "ag2" in phases:
            all_gather(2)
        if "e3" in phases:
            edge_phase(3)
        if "ag3" in phases:
            all_gather(3)
        if "e4" in phases:
            edge_phase(4)

        # ---------------- pooled AllReduce + MLP head (f32) ----------------
        fin_on = "fin" in phases
        ar_sb = cst.tile([HID, B + 1], F32, name="ar_sb", tag="ar_sb")
        if fin_on:
            nc.vector.memset(ar_sb[:], 0.0)
            nc.vector.tensor_copy(ar_sb[:, 0:B], psA[:])
            nc.vector.tensor_copy(ar_sb[0:B, B:B + 1], psB[:])
            nc.sync.dma_start(out=arin[:], in_=ar_sb[:])
            if n_cores == 1:
                nc.sync.dma_start(out=arout[:], in_=arin[:])
            else:
                nc.gpsimd.collective_compute(
                    "AllReduce", A.add, replica_groups=rg,
                    ins=[arin[:]], outs=[arout[:]])
            full = cst.tile([HID, B + 1], F32, name="arf", tag="arf")
            nc.sync.dma_start(out=full[:], in_=arout[:])
            cnt = cst.tile([B, 1], F32, name="cnt", tag="cnt")
            nc.vector.tensor_scalar(cnt[:], full[0:B, B:B + 1], 1.0, None,
                                    A.max)
            rcnt = cst.tile([B, 1], F32, name="rcnt", tag="rcnt")
            nc.vector.reciprocal(rcnt[:], cnt[:])
            z1p = tpool.tile([B, MH], F32, tag="tN", name="z1p")
            nc.tensor.matmul(z1p[:], lhsT=full[:, 0:B], rhs=wh1_s[:],
                             start=True, stop=True)
            z = cst.tile([B, MH], F32, name="z", tag="z")
            nc.vector.scalar_tensor_tensor(out=z[:], in0=z1p[:],
                                           scalar=rcnt[:, 0:1],
                                           in1=bh1rep_s[:],
                                           op0=A.mult, op1=A.add)
            nc.vector.tensor_scalar(z[:], z[:], 0.0, None, A.max)
            zps = tpool.tile([MH, B], F32, tag="tN", name="zps")
            nc.tensor.transpose(zps[:], z[:], idf32_s[0:B, 0:B])
            zT = cst.tile([MH, B], F32, name="zT", tag="zT")
            nc.vector.tensor_copy(zT[:], zps[:])
            ops_ = tpool.tile([B, C], F32, tag="tN", name="ops_")
            nc.tensor.matmul(ops_[:], lhsT=zT[:], rhs=wh2_s[:],
                             start=True, stop=True)
            o_sb = cst.tile([B, C], F32, name="o_sb", tag="o_sb")
            nc.vector.tensor_tensor(out=o_sb[:], in0=ops_[:],
                                    in1=bh2rep_s[:], op=A.add)
            nc.sync.dma_start(out=out_d[:], in_=o_sb[:])
        _freeB()
        _freeA()

    nc.compile()
    return nc


# ----------------------------------------------------------------------------
# Runner
# ----------------------------------------------------------------------------

def make_in_maps(meta, common, per_core):
    maps = []
    for pc in per_core:
        m = dict(common)
        m.update(pc)
        maps.append(m)
    return maps


def run(inputs, n_cores=N_CORES, half=None, G=DEF_G, B=None, trace=False,
        phases=None):
    from concourse.bass_utils import run_bass_kernel_spmd
    meta, common, per_core = preprocess(inputs, n_cores=n_cores, half=half,
                                        G=G, B=B)
    nc = build_program(meta, phases=phases)
    in_maps = make_in_maps(meta, common, per_core)
    res = run_bass_kernel_spmd(nc, in_maps, list(range(n_cores)), trace=trace)
    return res


def kernel(**inputs):
    res = run(inputs)
    return np.asarray(res.results[0]["out"], np.float32)
